# revision 11
# baseline (speedup 1.0000x reference)
"""Gaussian label-splat density kernel for Trainium2 (8 NeuronCores).

Math (matches the reference): for each batch b
    gx[n, w] = exp(-(w - lx[n])^2 / (2 sigma^2))   (normalized over w)
    gy[n, h] = exp(-(h - ly[n])^2 / (2 sigma^2))   (normalized over h)
    density[b, 0] = sum_n outer(gy[n], gx[n])      (K = 64 labels)

batch_images contributes only its shape, so the kernel never touches it.

Sharding: core c -> (batch b = c // 2, row half t = c % 2, h0 = 256 * t).
Each core builds its gaussians from a 2 KB label packet and emits a
(256, 512) output tile. No cross-core comms.

All matmul operands stay at partition base 0 (the PE rejects a nonzero
row tile position at runtime on this stack). The critical-path layout:

  - BOTH normalizers come from the truncated-tail identity
        sum_{w=0..511} = sigma*sqrt(2*pi) - left tail - right tail
    (Poisson summation; correction < 3e-9 for sigma >= 1), so no
    row-reduce of the materialized x profile is needed and the exp
    queue's output feeds a short (64,128) reduce per axis.
  - the x squares run on the otherwise-idle GpSimd (iota + add + mul),
    freeing the ACT queue; tail/slice squares run on Vector.
  - ACT queue order (pinned): warm-up -> x-tails exp -> y-tails exp ->
    slice exp (2 column halves) -> x exp (2 column halves). The
    normalizer chain drains first, the 2 lhsT halves next, and the
    matmul rhs arrives exactly when the PE needs each half.
  - 1/(Zx*Zy) folds into the slice via one tensor_scalar each per
    column half (the halves feed LDWEIGHTS for row blocks t=0,1).
  - 4 matmuls: per row block t, rhs column halves [0,256) and
    [256,512) (N=256 keeps f32r at 1 cycle/row); PSUM -> SBUF copies
    chase the matmuls at (128,256) granularity on Vector.

Output DMAs are issued OUTSIDE the TileContext: the tile-exit
all-engine barrier orders them after the copies, and nothing waits on
their completion semaphore -- the NEFF's fixed multi-microsecond
semaphore-reset epilogue (which the measured window includes anyway)
covers the DMA flight time, so the ~2.2us DMA completion latency
disappears from the critical path. The DMAs still carry a semaphore
increment (walrus requires sync info on DGE descriptors); since this
NEFF only ever increments it and nothing waits on it, a stale value
across executions is harmless.

An input-independent warm-up exp pulls the ~1.3us ACT table load into
the label-DMA completion window.

Label packet (built on host), partitions 0..63 = labels, 8 f32 cols:
    col 0 = -lx              (bias for the x square)
    col 1 = h0 - ly          (bias for the y slice diff)
    col 2 = lx + 1           (x left-tail offset)
    col 3 = 512 - lx         (x right-tail offset)
    col 4 = ly + 1           (y left-tail offset)
    col 5 = 512 - ly         (y right-tail offset)
    col 6 = -1/(2 sigma^2)   (exp scale)
    col 7 = sigma*sqrt(2pi)  (infinite-range gaussian sum)
"""

import numpy as np

import concourse.bacc as bacc
import concourse.tile as tile
from concourse.tile import add_dep_helper
from concourse import mybir
from concourse.bass_utils import run_bass_kernel_spmd

B, NLAB, H, W = 4, 64, 512, 512
P = 128  # output rows per matmul block
HALF = H // 2  # output rows per core
WH = W // 2  # matmul N-split
NTAIL = 64  # terms per truncation tail
N_CORES = 8
F32 = mybir.dt.float32
F32R = mybir.dt.float32r
SQRT_2PI = 2.5066282746310002

_CACHE: list = []
_DMA_IN_TILE = False  # debug toggle: tile-managed output DMAs vs raw post-tile
_RAW_STAGE = True  # debug toggle: raw sbuf staging vs pool tiles


def _build():
    AF = mybir.ActivationFunctionType
    AX = mybir.AxisListType
    nc = bacc.Bacc(
        "TRN2",
        debug=False,
        target_bir_lowering=False,
        num_devices=N_CORES,
        enable_partition_id=False,
    )
    labels = nc.dram_tensor("labels", (NLAB, 8), F32, kind="ExternalInput").ap()
    out = nc.dram_tensor("out", (HALF, W), F32, kind="ExternalOutput").ap()

    # raw (non-tile) staging buffers so the post-context DMAs can read them
    stage = [nc.alloc_sbuf_tensor(f"stage{t}", (P, W), F32) for t in range(2)]
    # completion sem for the fire-and-forget output DMAs (walrus requires
    # sync info on DGE); nothing ever waits on it
    dma_sem = nc.alloc_semaphore("out_dma_sem")

    with tile.TileContext(nc) as tc:
        with (
            tc.tile_pool(name="sb", bufs=1) as pool,
            tc.tile_pool(name="ob", bufs=2) as opool,
            tc.tile_pool(name="ps", bufs=2, space="PSUM") as psum,
        ):
            # input-independent warm-up op so the ACT_TABLE_LOAD lands here
            # and hides under the label DMA's completion latency
            warm = pool.tile([NLAB, 1], F32)
            nc.vector.memset(warm, 0.0)
            nc.scalar.activation(warm, warm, AF.Exp, scale=1.0)

            L = pool.tile([NLAB, 8], F32)
            nc.sync.dma_start(out=L, in_=labels)

            I = pool.tile([NLAB, W], F32)
            nc.gpsimd.iota(
                I,
                pattern=[[1, W]],
                base=0,
                channel_multiplier=0,
                allow_small_or_imprecise_dtypes=True,
            )

            # x squares on GpSimd (otherwise idle; keeps ACT free for exps)
            Dx = pool.tile([NLAB, W], F32)
            nc.gpsimd.tensor_scalar_add(Dx, I, L[:, 0:1])
            SQx = pool.tile([NLAB, W], F32)
            nc.gpsimd.tensor_mul(SQx, Dx, Dx)

            # DVE: tail diffs (4 groups of 64: xl, xr, yl, yr) + squares
            Dt = pool.tile([NLAB, 4 * NTAIL], F32)
            for k in range(4):
                nc.vector.tensor_scalar_add(
                    Dt[:, k * NTAIL : (k + 1) * NTAIL],
                    I[:, 0:NTAIL],
                    L[:, 2 + k : 3 + k],
                )
            SQt = pool.tile([NLAB, 4 * NTAIL], F32)
            nc.vector.tensor_mul(SQt, Dt, Dt)
            # slice diffs + squares
            Ds = pool.tile([NLAB, HALF], F32)
            nc.vector.tensor_scalar_add(Ds, I[:, 0:HALF], L[:, 1:2])
            SQs = pool.tile([NLAB, HALF], F32)
            nc.vector.tensor_mul(SQs, Ds, Ds)

            # ACT chain (order pinned): x-tails -> y-tails -> slice halves -> x halves
            Gt = pool.tile([NLAB, 4 * NTAIL], F32)
            i_etx = nc.scalar.activation(
                Gt[:, 0 : 2 * NTAIL], SQt[:, 0 : 2 * NTAIL], AF.Exp, scale=L[:, 6:7]
            )
            i_ety = nc.scalar.activation(
                Gt[:, 2 * NTAIL : 4 * NTAIL],
                SQt[:, 2 * NTAIL : 4 * NTAIL],
                AF.Exp,
                scale=L[:, 6:7],
            )
            Gs = pool.tile([NLAB, HALF], F32)
            i_es = [
                nc.scalar.activation(
                    Gs[:, P * t : P * (t + 1)],
                    SQs[:, P * t : P * (t + 1)],
                    AF.Exp,
                    scale=L[:, 6:7],
                )
                for t in range(2)
            ]
            Gx = pool.tile([NLAB, W], F32R)
            i_ex = [
                nc.scalar.activation(
                    Gx[:, WH * u : WH * (u + 1)],
                    SQx[:, WH * u : WH * (u + 1)],
                    AF.Exp,
                    scale=L[:, 6:7],
                )
                for u in range(2)
            ]
            chain = [i_etx, i_ety, i_es[0], i_es[1], i_ex[0], i_ex[1]]
            for a, b in zip(chain[1:], chain):
                add_dep_helper(a.ins, b.ins, sync=False, reason="ACT queue order")

            # normalizers: Z = S - (left tail + right tail), per axis
            xt = pool.tile([NLAB, 1], F32)
            nc.vector.reduce_sum(xt, Gt[:, 0 : 2 * NTAIL], axis=AX.X)
            yt = pool.tile([NLAB, 1], F32)
            nc.vector.reduce_sum(yt, Gt[:, 2 * NTAIL : 4 * NTAIL], axis=AX.X)
            Zx = pool.tile([NLAB, 1], F32)
            nc.gpsimd.tensor_sub(Zx, L[:, 7:8], xt)
            Zy = pool.tile([NLAB, 1], F32)
            nc.gpsimd.tensor_sub(Zy, L[:, 7:8], yt)
            Zxy = pool.tile([NLAB, 1], F32)
            nc.vector.tensor_mul(Zxy, Zx, Zy)
            Rxy = pool.tile([NLAB, 1], F32)
            nc.vector.reciprocal(Rxy, Zxy)

            # fold both normalizers into the lhsT, one half per row block
            GY = pool.tile([NLAB, HALF], F32R)
            for t in range(2):
                nc.vector.tensor_scalar_mul(
                    GY[:, P * t : P * (t + 1)], Gs[:, P * t : P * (t + 1)], Rxy
                )

            for t in range(2):
                acc = psum.tile([P, W], F32)
                st = stage[t].ap() if _RAW_STAGE else opool.tile([P, W], F32)
                for u in range(2):
                    nc.tensor.matmul(
                        acc[:, WH * u : WH * (u + 1)],
                        GY[:, P * t : P * (t + 1)],
                        Gx[:, WH * u : WH * (u + 1)],
                        start=True,
                        stop=True,
                    )
                    # copy each half as soon as its matmul retires
                    nc.vector.tensor_copy(
                        st[:, WH * u : WH * (u + 1)], acc[:, WH * u : WH * (u + 1)]
                    )
                if _DMA_IN_TILE:
                    nc.sync.dma_start(out=out[P * t : P * (t + 1), :], in_=st)

    if not _DMA_IN_TILE:
        for t in range(2):
            nc.sync.dma_start(
                out=out[P * t : P * (t + 1), :], in_=stage[t].ap()
            ).then_inc(dma_sem, 16)

    nc.compile()
    return nc


def _in_maps(batch_labels: np.ndarray, sigma: float) -> list:
    m = np.float32(-1.0 / (2.0 * sigma * sigma))
    s = np.float32(sigma * SQRT_2PI)
    maps = []
    for c in range(N_CORES):
        b, t = divmod(c, 2)
        h0 = t * HALF
        lx = batch_labels[b, :, 0]
        ly = batch_labels[b, :, 1]
        packed = np.zeros((NLAB, 8), np.float32)
        packed[:, 0] = -lx
        packed[:, 1] = h0 - ly
        packed[:, 2] = lx + 1.0
        packed[:, 3] = float(W) - lx
        packed[:, 4] = ly + 1.0
        packed[:, 5] = float(H) - ly
        packed[:, 6] = m
        packed[:, 7] = s
        maps.append({"labels": packed})
    return maps


def _get_nc():
    if not _CACHE:
        _CACHE.append(_build())
    return _CACHE[0]


def _gather(results) -> np.ndarray:
    density = np.empty((B, 1, H, W), np.float32)
    for c in range(N_CORES):
        b, t = divmod(c, 2)
        density[b, 0, t * HALF : (t + 1) * HALF, :] = results[c]["out"]
    return density


def kernel(batch_images, batch_labels, sigma) -> np.ndarray:
    batch_labels = np.asarray(batch_labels, dtype=np.float32)
    sigma = float(np.asarray(sigma))
    nc = _get_nc()
    res = run_bass_kernel_spmd(
        nc, _in_maps(batch_labels, sigma), core_ids=list(range(N_CORES))
    )
    return _gather(res.results)


# revision 12
# speedup vs baseline: 1.4560x; 1.4560x over previous
"""Gaussian label-splat density kernel for Trainium2 (8 NeuronCores).

Math (matches the reference): for each batch b
    gx[n, w] = exp(-(w - lx[n])^2 / (2 sigma^2))   (normalized over w)
    gy[n, h] = exp(-(h - ly[n])^2 / (2 sigma^2))   (normalized over h)
    density[b, 0] = sum_n outer(gy[n], gx[n])      (K = 64 labels)

batch_images contributes only its shape, so the kernel never touches it.

Sharding: core c -> (batch b = c // 2, row half t = c % 2, h0 = 256 * t).
Each core builds its gaussians from a 2 KB label packet and emits a
(256, 512) output tile. No cross-core comms.

Critical-path layout (all matmul operands at partition base 0 -- the PE
rejects a nonzero row tile position at runtime on this stack; and no
GpSimd tensor ops -- they run ucode at ~4 G elem/s AND stall concurrent
DVE work through SBUF contention, so GpSimd only does the iota):

  - BOTH normalizers come from the truncated-tail identity
        sum_{w=0..511} = sigma*sqrt(2*pi) - left tail - right tail
    (Poisson summation; correction < 3e-9 for sigma >= 1): no row-reduce
    of the materialized x profile, just two (64,128) reduces of a small
    tails tile, feeding 1/(Zx*Zy) which folds into the lhsT halves.
  - ACT queue order (pinned): warm-up -> x square -> x-tails exp ->
    y-tails exp -> slice exp halves -> x exp halves. The normalizer
    chain drains first, the two lhsT halves next, and each matmul rhs
    half lands exactly when the PE needs it.
  - 4 matmuls: per row block t, rhs column halves (N=256 keeps f32r at
    1 cycle/row); PSUM -> SBUF copies chase the matmuls at (128,256)
    granularity on Vector into ONE fused (128,1024) staging tensor.

The output DMA is issued OUTSIDE the TileContext as a single
instruction: the tile-exit all-engine barrier orders it after the
copies, and nothing waits on its completion semaphore -- the NEFF's
fixed multi-microsecond semaphore-reset epilogue (inside the measured
window anyway) covers the DMA flight time, so the ~2.2us DMA completion
latency disappears from the critical path. The DMA carries a semaphore
increment (walrus requires sync info on DGE); since this NEFF only ever
increments it and nothing waits on it, a stale value across executions
is harmless. The DRAM output is declared (2, 128, 512) (same memory
layout as (256, 512)) so one DMA with a transposed dst access pattern
covers both row blocks from the fused staging tensor.

An input-independent warm-up exp pulls the ~1.3us ACT table load into
the label-DMA completion window.

Label packet (built on host), partitions 0..63 = labels, 8 f32 cols:
    col 0 = -lx              (bias for the x square)
    col 1 = h0 - ly          (bias for the y slice diff)
    col 2 = lx + 1           (x left-tail offset)
    col 3 = 512 - lx         (x right-tail offset)
    col 4 = ly + 1           (y left-tail offset)
    col 5 = 512 - ly         (y right-tail offset)
    col 6 = -1/(2 sigma^2)   (exp scale)
    col 7 = sigma*sqrt(2pi)  (infinite-range gaussian sum)
"""

import numpy as np

import concourse.bacc as bacc
import concourse.tile as tile
from concourse.tile import add_dep_helper
from concourse import mybir
from concourse.bass_utils import run_bass_kernel_spmd

B, NLAB, H, W = 4, 64, 512, 512
P = 128  # output rows per matmul block
HALF = H // 2  # output rows per core
WH = W // 2  # matmul N-split
NTAIL = 64  # terms per truncation tail
N_CORES = 8
F32 = mybir.dt.float32
F32R = mybir.dt.float32r
SQRT_2PI = 2.5066282746310002

_CACHE: list = []


def _build():
    AF = mybir.ActivationFunctionType
    AX = mybir.AxisListType
    nc = bacc.Bacc(
        "TRN2",
        debug=False,
        target_bir_lowering=False,
        num_devices=N_CORES,
        enable_partition_id=False,
    )
    labels = nc.dram_tensor("labels", (NLAB, 8), F32, kind="ExternalInput").ap()
    # (2, 128, 512) has the same memory layout as (256, 512); the split
    # shape lets one DMA cover both row blocks from the fused staging
    out = nc.dram_tensor("out", (2, P, W), F32, kind="ExternalOutput").ap()

    # raw (non-tile) staging so the post-context DMA can read it:
    # cols [512t + 256u, +256) = row block t, column half u
    stage = nc.alloc_sbuf_tensor("stage", (P, 2 * W), F32)
    # completion sem for the fire-and-forget output DMA (walrus requires
    # sync info on DGE); nothing ever waits on it
    dma_sem = nc.alloc_semaphore("out_dma_sem")

    with tile.TileContext(nc) as tc:
        with (
            tc.tile_pool(name="sb", bufs=1) as pool,
            tc.tile_pool(name="ps", bufs=2, space="PSUM") as psum,
        ):
            # input-independent warm-up op so the ACT_TABLE_LOAD lands here
            # and hides under the label DMA's completion latency
            warm = pool.tile([NLAB, 1], F32)
            nc.vector.memset(warm, 0.0)
            nc.scalar.activation(warm, warm, AF.Exp, scale=1.0)

            L = pool.tile([NLAB, 8], F32)
            nc.sync.dma_start(out=L, in_=labels)

            I = pool.tile([NLAB, W], F32)
            nc.gpsimd.iota(
                I,
                pattern=[[1, W]],
                base=0,
                channel_multiplier=0,
                allow_small_or_imprecise_dtypes=True,
            )

            # DVE: tail diffs (4 groups of 64: xl, xr, yl, yr) + squares
            Dt = pool.tile([NLAB, 4 * NTAIL], F32)
            for k in range(4):
                nc.vector.tensor_scalar_add(
                    Dt[:, k * NTAIL : (k + 1) * NTAIL],
                    I[:, 0:NTAIL],
                    L[:, 2 + k : 3 + k],
                )
            SQt = pool.tile([NLAB, 4 * NTAIL], F32)
            nc.vector.tensor_mul(SQt, Dt, Dt)
            # slice diffs + squares
            Ds = pool.tile([NLAB, HALF], F32)
            nc.vector.tensor_scalar_add(Ds, I[:, 0:HALF], L[:, 1:2])
            SQs = pool.tile([NLAB, HALF], F32)
            nc.vector.tensor_mul(SQs, Ds, Ds)

            # ACT chain (order pinned): x square -> x/y tails -> slice -> x
            SQx = pool.tile([NLAB, W], F32)
            i_sq = nc.scalar.activation(SQx, I, AF.Square, bias=L[:, 0:1], scale=1.0)
            Gt = pool.tile([NLAB, 4 * NTAIL], F32)
            i_etx = nc.scalar.activation(
                Gt[:, 0 : 2 * NTAIL], SQt[:, 0 : 2 * NTAIL], AF.Exp, scale=L[:, 6:7]
            )
            i_ety = nc.scalar.activation(
                Gt[:, 2 * NTAIL : 4 * NTAIL],
                SQt[:, 2 * NTAIL : 4 * NTAIL],
                AF.Exp,
                scale=L[:, 6:7],
            )
            Gs = pool.tile([NLAB, HALF], F32)
            i_es = [
                nc.scalar.activation(
                    Gs[:, P * t : P * (t + 1)],
                    SQs[:, P * t : P * (t + 1)],
                    AF.Exp,
                    scale=L[:, 6:7],
                )
                for t in range(2)
            ]
            Gx = pool.tile([NLAB, W], F32R)
            i_ex = [
                nc.scalar.activation(
                    Gx[:, WH * u : WH * (u + 1)],
                    SQx[:, WH * u : WH * (u + 1)],
                    AF.Exp,
                    scale=L[:, 6:7],
                )
                for u in range(2)
            ]
            chain = [i_sq, i_etx, i_ety, i_es[0], i_es[1], i_ex[0], i_ex[1]]
            for a, b in zip(chain[1:], chain):
                add_dep_helper(a.ins, b.ins, sync=False, reason="ACT queue order")

            # normalizers: Z = S - (left tail + right tail), per axis
            xt = pool.tile([NLAB, 1], F32)
            nc.vector.reduce_sum(xt, Gt[:, 0 : 2 * NTAIL], axis=AX.X)
            yt = pool.tile([NLAB, 1], F32)
            nc.vector.reduce_sum(yt, Gt[:, 2 * NTAIL : 4 * NTAIL], axis=AX.X)
            Zx = pool.tile([NLAB, 1], F32)
            nc.vector.tensor_sub(Zx, L[:, 7:8], xt)
            Zy = pool.tile([NLAB, 1], F32)
            nc.vector.tensor_sub(Zy, L[:, 7:8], yt)
            Zxy = pool.tile([NLAB, 1], F32)
            nc.vector.tensor_mul(Zxy, Zx, Zy)
            Rxy = pool.tile([NLAB, 1], F32)
            nc.vector.reciprocal(Rxy, Zxy)

            # fold both normalizers into the lhsT, one half per row block
            GY = pool.tile([NLAB, HALF], F32R)
            for t in range(2):
                nc.vector.tensor_scalar_mul(
                    GY[:, P * t : P * (t + 1)], Gs[:, P * t : P * (t + 1)], Rxy
                )

            st = stage.ap()
            for t in range(2):
                acc = psum.tile([P, W], F32)
                for u in range(2):
                    nc.tensor.matmul(
                        acc[:, WH * u : WH * (u + 1)],
                        GY[:, P * t : P * (t + 1)],
                        Gx[:, WH * u : WH * (u + 1)],
                        start=True,
                        stop=True,
                    )
                    # copy each half as soon as its matmul retires
                    nc.vector.tensor_copy(
                        st[:, W * t + WH * u : W * t + WH * (u + 1)],
                        acc[:, WH * u : WH * (u + 1)],
                    )

    # single fire-and-forget output DMA: partition p, col 512t+c ->
    # out[t, p, c]; ordered after the copies by the tile-exit barrier
    nc.sync.dma_start(out=out.transpose([1, 0, 2]), in_=stage.ap()).then_inc(
        dma_sem, 16
    )

    nc.compile()
    return nc


def _in_maps(batch_labels: np.ndarray, sigma: float) -> list:
    m = np.float32(-1.0 / (2.0 * sigma * sigma))
    s = np.float32(sigma * SQRT_2PI)
    maps = []
    for c in range(N_CORES):
        b, t = divmod(c, 2)
        h0 = t * HALF
        lx = batch_labels[b, :, 0]
        ly = batch_labels[b, :, 1]
        packed = np.zeros((NLAB, 8), np.float32)
        packed[:, 0] = -lx
        packed[:, 1] = h0 - ly
        packed[:, 2] = lx + 1.0
        packed[:, 3] = float(W) - lx
        packed[:, 4] = ly + 1.0
        packed[:, 5] = float(H) - ly
        packed[:, 6] = m
        packed[:, 7] = s
        maps.append({"labels": packed})
    return maps


def _get_nc():
    if not _CACHE:
        _CACHE.append(_build())
    return _CACHE[0]


def _gather(results) -> np.ndarray:
    density = np.empty((B, 1, H, W), np.float32)
    for c in range(N_CORES):
        b, t = divmod(c, 2)
        density[b, 0, t * HALF : (t + 1) * HALF, :] = results[c]["out"].reshape(
            HALF, W
        )
    return density


def kernel(batch_images, batch_labels, sigma) -> np.ndarray:
    batch_labels = np.asarray(batch_labels, dtype=np.float32)
    sigma = float(np.asarray(sigma))
    nc = _get_nc()
    res = run_bass_kernel_spmd(
        nc, _in_maps(batch_labels, sigma), core_ids=list(range(N_CORES))
    )
    return _gather(res.results)


# revision 13
# speedup vs baseline: 1.5630x; 1.0735x over previous
"""Gaussian label-splat density kernel for Trainium2 (8 NeuronCores).

Math (matches the reference): for each batch b
    gx[n, w] = exp(-(w - lx[n])^2 / (2 sigma^2))   (normalized over w)
    gy[n, h] = exp(-(h - ly[n])^2 / (2 sigma^2))   (normalized over h)
    density[b, 0] = sum_n outer(gy[n], gx[n]) = gy.T @ gx    (K = 64 labels)

batch_images contributes only its shape, so the kernel never touches it.

Sharding: core c -> (batch b = c // 2, row half t = c % 2, h0 = 256 * t).
Each core builds its own gaussians from a 2 KB label packet and emits a
(256, 512) output tile as two 128x512 matmuls. No cross-core comms.

Compute core (measured-best: few big ops beat many small ones -- each
extra op costs ~150 ns fixed plus ~100-150 ns semaphore handoff):
the x profile is materialized in full (matmul rhs) and Zx is a row-sum
of it. The y profile is only needed through its normalizer Zy and a
256-row slice: Zy comes from the exact split sum_{h in Z} - left tail -
right tail, where the lattice sum is sigma*sqrt(2*pi) (Poisson
summation; correction < 3e-9 for sigma >= 1) and both 64-term tails fit
one small (64,128) exp with accum_out. Both normalizers (1/Zx * 1/Zy)
fold into the y-slice halves (lhsT) via one dual-scalar op each;
matmuls run in f32r. An input-independent warm-up exp pulls the ~1.3us
ACT table load into the label-DMA completion window. The store path
(PSUM->SBUF copies) stays on Vector.

Output path: copies land in ONE fused raw (128, 1024) staging tensor
(cols [512t, 512t+512) = row block t), and a SINGLE output DMA is
issued OUTSIDE the TileContext: the tile-exit all-engine barrier orders
it after the copies, and nothing waits on its completion semaphore --
the NEFF's fixed multi-microsecond semaphore-reset epilogue (inside the
measured window anyway) covers the DMA flight time, so the ~2.2us DMA
completion latency disappears from the critical path. The DMA carries a
semaphore increment (walrus requires sync info on DGE); nothing waits
on it, and since this NEFF only ever increments it, a stale value
across executions is harmless. The DRAM output is declared (2,128,512)
(same memory layout as (256,512)) so the one DMA with a transposed dst
access pattern covers both row blocks.

Label packet (built on host), partitions 0..63 = labels, 8 f32 cols:
    col 0 = -lx              (bias for the x square)
    col 1 = h0 - ly          (bias for the y row-window square)
    col 2 = ly + 1           (left-tail offset)
    col 3 = 512 - ly         (right-tail offset)
    col 4 = -1/(2 sigma^2)   (exp scale)
    col 5 = sigma*sqrt(2pi)  (infinite-range gaussian sum)
"""

import numpy as np

import concourse.bacc as bacc
import concourse.tile as tile
from concourse.tile import add_dep_helper
from concourse import mybir
from concourse.bass_utils import run_bass_kernel_spmd

B, NLAB, H, W = 4, 64, 512, 512
P = 128
HALF = H // 2  # output rows per core
NTAIL = 64  # terms per truncation tail
N_CORES = 8
F32 = mybir.dt.float32
F32R = mybir.dt.float32r
SQRT_2PI = 2.5066282746310002

_CACHE: list = []


def _build():
    AF = mybir.ActivationFunctionType
    AX = mybir.AxisListType
    OP = mybir.AluOpType
    nc = bacc.Bacc(
        "TRN2",
        debug=False,
        target_bir_lowering=False,
        num_devices=N_CORES,
        enable_partition_id=False,
    )
    labels = nc.dram_tensor("labels", (NLAB, 8), F32, kind="ExternalInput").ap()
    # (2, 128, 512) has the same memory layout as (256, 512); the split
    # shape lets one DMA cover both row blocks from the fused staging
    out = nc.dram_tensor("out", (2, P, W), F32, kind="ExternalOutput").ap()

    # raw (non-tile) staging so the post-context DMA can read it
    stage = nc.alloc_sbuf_tensor("stage", (P, 2 * W), F32)
    # completion sem for the fire-and-forget output DMA (walrus requires
    # sync info on DGE); nothing ever waits on it
    dma_sem = nc.alloc_semaphore("out_dma_sem")

    with tile.TileContext(nc) as tc:
        with (
            tc.tile_pool(name="sb", bufs=1) as pool,
            tc.tile_pool(name="ps", bufs=2, space="PSUM") as psum,
        ):
            # input-independent warm-up op so the ACT_TABLE_LOAD lands here
            # and hides under the label DMA's completion latency
            warm = pool.tile([NLAB, 1], F32)
            nc.vector.memset(warm, 0.0)
            nc.scalar.activation(warm, warm, AF.Exp, scale=1.0)

            L = pool.tile([NLAB, 8], F32)
            nc.sync.dma_start(out=L, in_=labels)

            I = pool.tile([NLAB, W], F32)
            nc.gpsimd.iota(
                I,
                pattern=[[1, W]],
                base=0,
                channel_multiplier=0,
                allow_small_or_imprecise_dtypes=True,
            )

            # x square on ACT, then the full x profile (matmul rhs, f32r)
            SQx = pool.tile([NLAB, W], F32)
            i_sqx = nc.scalar.activation(SQx, I, AF.Square, bias=L[:, 0:1], scale=1.0)
            Gx = pool.tile([NLAB, W], F32R)
            i_ex = nc.scalar.activation(Gx, SQx, AF.Exp, scale=L[:, 4:5])
            Zx = pool.tile([NLAB, 1], F32)
            nc.vector.reduce_sum(Zx, Gx, axis=AX.X)
            Rx = pool.tile([NLAB, 1], F32)
            nc.vector.reciprocal(Rx, Zx)

            # y truncation tails: cols 0..63 = j + (ly+1), 64..127 = j + (512-ly)
            Dt = pool.tile([NLAB, 2 * NTAIL], F32)
            nc.vector.tensor_scalar_add(Dt[:, 0:NTAIL], I[:, 0:NTAIL], L[:, 2:3])
            nc.vector.tensor_scalar_add(
                Dt[:, NTAIL : 2 * NTAIL], I[:, 0:NTAIL], L[:, 3:4]
            )
            SQt = pool.tile([NLAB, 2 * NTAIL], F32)
            nc.vector.tensor_mul(SQt, Dt, Dt)
            Gt = pool.tile([NLAB, 2 * NTAIL], F32)
            Tsum = pool.tile([NLAB, 1], F32)
            i_et = nc.scalar.activation(
                Gt, SQt, AF.Exp, scale=L[:, 4:5], accum_out=Tsum
            )
            # the subtract runs on the otherwise-idle GpSimd so the Vector
            # queue (row-sum -> reciprocals -> normalize) stays short
            Zy = pool.tile([NLAB, 1], F32)
            nc.gpsimd.tensor_sub(Zy, L[:, 5:6], Tsum)

            # y slice square (DVE) + exp (ACT)
            Ds = pool.tile([NLAB, HALF], F32)
            nc.vector.tensor_scalar_add(Ds, I[:, 0:HALF], L[:, 1:2])
            SQs = pool.tile([NLAB, HALF], F32)
            nc.vector.tensor_mul(SQs, Ds, Ds)
            Gs = pool.tile([NLAB, HALF], F32)
            i_es = nc.scalar.activation(Gs, SQs, AF.Exp, scale=L[:, 4:5])
            # pin the ACT queue order: SQx -> Ex -> tails-exp -> slice-exp, so
            # the x chain (which feeds the long DVE row-sum) never slips
            add_dep_helper(i_et.ins, i_ex.ins, sync=False, reason="ACT order: tails after Ex")
            add_dep_helper(i_es.ins, i_et.ins, sync=False, reason="ACT order: slice last")

            Ry = pool.tile([NLAB, 1], F32)
            nc.vector.reciprocal(Ry, Zy)

            # both normalizers fold into the small lhsT in one dual-scalar op
            # per half; rhs = Gx raw. Halved so the first LDWEIGHTS can start
            # sooner.
            GYn = pool.tile([NLAB, HALF], F32R)
            nc.vector.tensor_scalar(
                GYn[:, 0:P], Gs[:, 0:P], Rx, Ry, OP.mult, OP.mult
            )
            nc.vector.tensor_scalar(
                GYn[:, P:HALF], Gs[:, P:HALF], Rx, Ry, OP.mult, OP.mult
            )

            st = stage.ap()
            for t in range(2):
                acc = psum.tile([P, W], F32)
                nc.tensor.matmul(
                    acc,
                    GYn[:, t * P : (t + 1) * P],
                    Gx,
                    start=True,
                    stop=True,
                )
                nc.vector.tensor_copy(st[:, W * t : W * (t + 1)], acc)

    # single fire-and-forget output DMA: partition p, col 512t+c ->
    # out[t, p, c]; ordered after the copies by the tile-exit barrier
    nc.sync.dma_start(out=out.transpose([1, 0, 2]), in_=stage.ap()).then_inc(
        dma_sem, 16
    )

    nc.compile()
    return nc


def _in_maps(batch_labels: np.ndarray, sigma: float) -> list:
    m = np.float32(-1.0 / (2.0 * sigma * sigma))
    s = np.float32(sigma * SQRT_2PI)
    maps = []
    for c in range(N_CORES):
        b, t = divmod(c, 2)
        h0 = t * HALF
        lx = batch_labels[b, :, 0]
        ly = batch_labels[b, :, 1]
        packed = np.zeros((NLAB, 8), np.float32)
        packed[:, 0] = -lx
        packed[:, 1] = h0 - ly
        packed[:, 2] = ly + 1.0
        packed[:, 3] = float(H) - ly
        packed[:, 4] = m
        packed[:, 5] = s
        maps.append({"labels": packed})
    return maps


def _get_nc():
    if not _CACHE:
        _CACHE.append(_build())
    return _CACHE[0]


def _gather(results) -> np.ndarray:
    density = np.empty((B, 1, H, W), np.float32)
    for c in range(N_CORES):
        b, t = divmod(c, 2)
        density[b, 0, t * HALF : (t + 1) * HALF, :] = results[c]["out"].reshape(
            HALF, W
        )
    return density


def kernel(batch_images, batch_labels, sigma) -> np.ndarray:
    batch_labels = np.asarray(batch_labels, dtype=np.float32)
    sigma = float(np.asarray(sigma))
    nc = _get_nc()
    res = run_bass_kernel_spmd(
        nc, _in_maps(batch_labels, sigma), core_ids=list(range(N_CORES))
    )
    return _gather(res.results)


# revision 18
# speedup vs baseline: 1.5801x; 1.0110x over previous
"""Gaussian label-splat density kernel for Trainium2 (8 NeuronCores).

Math (matches the reference): for each batch b
    gx[n, w] = exp(-(w - lx[n])^2 / (2 sigma^2))   (normalized over w)
    gy[n, h] = exp(-(h - ly[n])^2 / (2 sigma^2))   (normalized over h)
    density[b, 0] = sum_n outer(gy[n], gx[n]) = gy.T @ gx    (K = 64 labels)

batch_images contributes only its shape, so the kernel never touches it.

Sharding: core c -> (batch b = c // 2, row half t = c % 2, h0 = 256 * t).
Each core builds its own gaussians from a 2 KB label packet and emits a
(256, 512) output tile as two 128x512 matmuls. No cross-core comms.

Compute core (measured-best: few big ops beat many small ones -- each
extra op costs ~150 ns fixed plus ~100-150 ns semaphore handoff):
the x profile is materialized in full (matmul rhs) and Zx is a row-sum
of it. The y profile is only needed through its normalizer Zy and a
256-row slice: Zy comes from the exact split sum_{h in Z} - left tail -
right tail, where the lattice sum is sigma*sqrt(2*pi) (Poisson
summation; correction < 3e-9 for sigma >= 1) and both 64-term tails fit
one small (64,128) exp with accum_out. Both normalizers (1/Zx * 1/Zy)
fold into the y-slice halves (lhsT) via one dual-scalar op each;
matmuls run in f32r. An input-independent warm-up exp pulls the ~1.3us
ACT table load into the label-DMA completion window. The store path
(PSUM->SBUF copies) stays on Vector.

Output path: copies land in ONE fused raw (128, 1024) staging tensor
(cols [512t, 512t+512) = row block t), and a SINGLE output DMA is
issued OUTSIDE the TileContext: the tile-exit all-engine barrier orders
it after the copies, and nothing waits on its completion semaphore --
the NEFF's fixed multi-microsecond semaphore-reset epilogue (inside the
measured window anyway) covers the DMA flight time, so the ~2.2us DMA
completion latency disappears from the critical path. The DMA carries a
semaphore increment (walrus requires sync info on DGE); nothing waits
on it, and since this NEFF only ever increments it, a stale value
across executions is harmless. The DRAM output is declared (2,128,512)
(same memory layout as (256,512)) so the one DMA with a transposed dst
access pattern covers both row blocks.

Label packet (built on host), partitions 0..63 = labels, 8 f32 cols:
    col 0 = -lx              (bias for the x square)
    col 1 = h0 - ly          (bias for the y row-window square)
    col 2 = ly + 1           (left-tail offset)
    col 3 = 512 - ly         (right-tail offset)
    col 4 = -1/(2 sigma^2)   (exp scale)
    col 5 = sigma*sqrt(2pi)  (infinite-range gaussian sum)
"""

import numpy as np

import concourse.bacc as bacc
import concourse.tile as tile
from concourse.tile import add_dep_helper
from concourse import mybir
from concourse.bass_utils import run_bass_kernel_spmd

B, NLAB, H, W = 4, 64, 512, 512
P = 128
HALF = H // 2  # output rows per core
NTAIL = 64  # terms per truncation tail
N_CORES = 8
F32 = mybir.dt.float32
F32R = mybir.dt.float32r
SQRT_2PI = 2.5066282746310002

_CACHE: list = []


def _build():
    AF = mybir.ActivationFunctionType
    AX = mybir.AxisListType
    OP = mybir.AluOpType
    nc = bacc.Bacc(
        "TRN2",
        debug=False,
        target_bir_lowering=False,
        num_devices=N_CORES,
        enable_partition_id=False,
    )
    labels = nc.dram_tensor("labels", (NLAB, 8), F32, kind="ExternalInput").ap()
    # row-interleaved output: matmul block t covers rows 2j + t, so SBUF
    # partition p holds DRAM rows 2p (cols 0:512) and 2p+1 (cols 512:1024)
    # = one contiguous 4 KB run per partition; (128, 1024) reshapes to the
    # (256, 512) tile on the host for free
    out = nc.dram_tensor("out", (P, 2 * W), F32, kind="ExternalOutput").ap()

    # raw (non-tile) staging so the post-context DMA can read it
    stage = nc.alloc_sbuf_tensor("stage", (P, 2 * W), F32)
    # completion sem for the fire-and-forget output DMA (walrus requires
    # sync info on DGE); nothing ever waits on it
    dma_sem = nc.alloc_semaphore("out_dma_sem")

    with tile.TileContext(nc) as tc:
        with (
            tc.tile_pool(name="sb", bufs=1) as pool,
            tc.tile_pool(name="ps", bufs=2, space="PSUM") as psum,
        ):
            # input-independent warm-up op so the ACT_TABLE_LOAD lands here
            # and hides under the label DMA's completion latency
            warm = pool.tile([NLAB, 1], F32)
            nc.vector.memset(warm, 0.0)
            nc.scalar.activation(warm, warm, AF.Exp, scale=1.0)

            L = pool.tile([NLAB, 8], F32)
            nc.sync.dma_start(out=L, in_=labels)

            I = pool.tile([NLAB, W], F32)
            nc.gpsimd.iota(
                I,
                pattern=[[1, W]],
                base=0,
                channel_multiplier=0,
                allow_small_or_imprecise_dtypes=True,
            )
            # slice iota, row-interleaved: cols [128t + j] = 2j + t, so the
            # lhsT for block t covers output rows h0 + 2j + t
            Iy = pool.tile([NLAB, HALF], F32)
            nc.gpsimd.iota(
                Iy,
                pattern=[[1, 2], [2, P]],
                base=0,
                channel_multiplier=0,
                allow_small_or_imprecise_dtypes=True,
            )

            # x square on ACT, then the full x profile (matmul rhs, f32r)
            SQx = pool.tile([NLAB, W], F32)
            i_sqx = nc.scalar.activation(SQx, I, AF.Square, bias=L[:, 0:1], scale=1.0)
            Gx = pool.tile([NLAB, W], F32R)
            i_ex = nc.scalar.activation(Gx, SQx, AF.Exp, scale=L[:, 4:5])
            Zx = pool.tile([NLAB, 1], F32)
            nc.vector.reduce_sum(Zx, Gx, axis=AX.X)
            Rx = pool.tile([NLAB, 1], F32)
            i_rx = nc.vector.reciprocal(Rx, Zx)

            # y truncation tails: cols 0..63 = j + (ly+1), 64..127 = j + (512-ly)
            Dt = pool.tile([NLAB, 2 * NTAIL], F32)
            nc.vector.tensor_scalar_add(Dt[:, 0:NTAIL], I[:, 0:NTAIL], L[:, 2:3])
            nc.vector.tensor_scalar_add(
                Dt[:, NTAIL : 2 * NTAIL], I[:, 0:NTAIL], L[:, 3:4]
            )
            SQt = pool.tile([NLAB, 2 * NTAIL], F32)
            nc.vector.tensor_mul(SQt, Dt, Dt)
            Gt = pool.tile([NLAB, 2 * NTAIL], F32)
            Tsum = pool.tile([NLAB, 1], F32)
            i_et = nc.scalar.activation(
                Gt, SQt, AF.Exp, scale=L[:, 4:5], accum_out=Tsum
            )
            # the subtract runs on the otherwise-idle GpSimd so the Vector
            # queue (row-sum -> reciprocals -> normalize) stays short
            Zy = pool.tile([NLAB, 1], F32)
            nc.gpsimd.tensor_sub(Zy, L[:, 5:6], Tsum)

            # y slice square (DVE) + exp (ACT)
            Ds = pool.tile([NLAB, HALF], F32)
            nc.vector.tensor_scalar_add(Ds, Iy, L[:, 1:2])
            SQs = pool.tile([NLAB, HALF], F32)
            nc.vector.tensor_mul(SQs, Ds, Ds)
            Gs = pool.tile([NLAB, HALF], F32)
            i_es = nc.scalar.activation(Gs, SQs, AF.Exp, scale=L[:, 4:5])
            # pin the ACT queue order: SQx -> Ex -> tails-exp -> slice-exp, so
            # the x chain (which feeds the long DVE row-sum) never slips
            add_dep_helper(i_et.ins, i_ex.ins, sync=False, reason="ACT order: tails after Ex")
            add_dep_helper(i_es.ins, i_et.ins, sync=False, reason="ACT order: slice last")

            Ry = pool.tile([NLAB, 1], F32)
            i_ry = nc.vector.reciprocal(Ry, Zy)
            # keep the Vector queue in data-arrival order: Rx's input (the
            # Gx row-sum) lands before Zy, so Rx must not queue behind Ry
            add_dep_helper(i_ry.ins, i_rx.ins, sync=False, reason="V order: Rx first")

            # both normalizers fold into the small lhsT in one dual-scalar op
            # per half; rhs = Gx raw. Halved so the first LDWEIGHTS can start
            # sooner.
            GYn = pool.tile([NLAB, HALF], F32R)
            nc.vector.tensor_scalar(
                GYn[:, 0:P], Gs[:, 0:P], Rx, Ry, OP.mult, OP.mult
            )
            nc.vector.tensor_scalar(
                GYn[:, P:HALF], Gs[:, P:HALF], Rx, Ry, OP.mult, OP.mult
            )

            st = stage.ap()
            for t in range(2):
                acc = psum.tile([P, W], F32)
                nc.tensor.matmul(
                    acc,
                    GYn[:, t * P : (t + 1) * P],
                    Gx,
                    start=True,
                    stop=True,
                )
                nc.vector.tensor_copy(st[:, W * t : W * (t + 1)], acc)

    # single fire-and-forget output DMA (identical src/dst patterns, one
    # contiguous 4 KB run per partition); ordered after the copies by the
    # tile-exit barrier
    nc.sync.dma_start(out=out, in_=stage.ap()).then_inc(dma_sem, 16)

    nc.compile()
    return nc


def _in_maps(batch_labels: np.ndarray, sigma: float) -> list:
    m = np.float32(-1.0 / (2.0 * sigma * sigma))
    s = np.float32(sigma * SQRT_2PI)
    maps = []
    for c in range(N_CORES):
        b, t = divmod(c, 2)
        h0 = t * HALF
        lx = batch_labels[b, :, 0]
        ly = batch_labels[b, :, 1]
        packed = np.zeros((NLAB, 8), np.float32)
        packed[:, 0] = -lx
        packed[:, 1] = h0 - ly
        packed[:, 2] = ly + 1.0
        packed[:, 3] = float(H) - ly
        packed[:, 4] = m
        packed[:, 5] = s
        maps.append({"labels": packed})
    return maps


def _get_nc():
    if not _CACHE:
        _CACHE.append(_build())
    return _CACHE[0]


def _gather(results) -> np.ndarray:
    density = np.empty((B, 1, H, W), np.float32)
    for c in range(N_CORES):
        b, t = divmod(c, 2)
        # (128, 1024) -> rows (2p, 2p+1): a plain reshape deinterleaves
        density[b, 0, t * HALF : (t + 1) * HALF, :] = results[c]["out"].reshape(
            HALF, W
        )
    return density


def kernel(batch_images, batch_labels, sigma) -> np.ndarray:
    batch_labels = np.asarray(batch_labels, dtype=np.float32)
    sigma = float(np.asarray(sigma))
    nc = _get_nc()
    res = run_bass_kernel_spmd(
        nc, _in_maps(batch_labels, sigma), core_ids=list(range(N_CORES))
    )
    return _gather(res.results)
